# revision 17
# baseline (speedup 1.0000x reference)
"""Trainium2 Bass kernel for nn_Attention (B=4, T=2048, C=1024, H=16, D=64).

Sharding: tensor-parallel over heads — 2 heads per core x 8 cores.
Each core computes:
  1. qkv projection for its 384 rows of Wqkv (x is pre-transposed + bf16 on host)
  2. causal attention for its 2 heads x 4 batches (S^T = K @ Q^T formulation,
     unnormalized softmax with the denominator computed via a ones-column in V)
  3. partial output projection (its 128 columns of the o-feature contraction)
Host sums the 8 partial outputs and adds bout.

v2: bf16 operands everywhere (fp32 PSUM accumulation), causal mask folded into
the S PSUM accumulation via an identity-matmul bias add, V' built with DMA-XBAR
transposes, proj evacuations on the Pool engine, out-proj evac on DVE.
"""

import os
import sys

import numpy as np

for _p in ("/opt/trn_rl_repo", "/root/.axon_site/_ro/trn_rl_repo"):
    if os.path.isdir(_p) and _p not in sys.path:
        sys.path.insert(0, _p)

import ml_dtypes  # noqa: E402
import concourse.tile as tile  # noqa: E402
from concourse import bacc, mybir  # noqa: E402
from concourse.bass_utils import run_bass_kernel_spmd  # noqa: E402

B, T, C = 4, 2048, 1024
H = 16
D = C // H  # 64
NCORES = 8
HPC = H // NCORES  # heads per core = 2
BT = B * T  # 8192
KT = 128  # k-tile (S^T partition dim)
QB = 512  # q-block (S^T free dim)
NKT = T // KT  # 16 k-tiles per batch
NQB = T // QB  # 4 q-blocks per batch
SCALE = 1.0 / np.sqrt(D)
MASK_BIG = 30000.0

F32 = mybir.dt.float32
BF16 = mybir.dt.bfloat16
BF16NP = ml_dtypes.bfloat16

ALLOWED, CAUSAL, GENERAL = 0, 1, 2


def _classify_mask(mask2d):
    """Per (q-block j, k-tile kt) classification, shared across (b, h).

    mask2d: [T, T] int32, mask2d[q, k] == 0 -> masked.
    Returns (plan, genbias):
      plan[j] = list of (kt, type, aux); skipped tiles omitted.
        aux = causal offset for CAUSAL, genbias index for GENERAL.
      genbias: [n_gen, 128, 512] additive bias in [k, q] orientation.
    """
    plan = [[] for _ in range(NQB)]
    gen = []
    for j in range(NQB):
        q0 = j * QB
        for kt in range(NKT):
            k0 = kt * KT
            sub = mask2d[q0 : q0 + QB, k0 : k0 + KT] != 0  # [q, k]
            if not sub.any():
                continue
            if sub.all():
                plan[j].append((kt, ALLOWED, 0))
                continue
            qi = np.arange(q0, q0 + QB)[:, None]
            ki = np.arange(k0, k0 + KT)[None, :]
            off = k0 - q0
            if off in (0, 128, 256, 384) and bool((sub == (qi >= ki)).all()):
                plan[j].append((kt, CAUSAL, off))
            else:
                bias = np.where(sub, 0.0, -MASK_BIG).astype(np.float32).T  # [k, q]
                gen.append(np.ascontiguousarray(bias))
                plan[j].append((kt, GENERAL, len(gen) - 1))
    genbias = np.stack(gen) if gen else np.zeros((1, KT, QB), np.float32)
    return plan, genbias


def _build_program(plan, n_gen, loop_n=1, phases=("proj", "attn", "out")):
    """Build the single-core Bass program (identical across cores).

    loop_n > 1 wraps the compute in a hardware loop (benchmarking only).
    """
    nc = bacc.Bacc("TRN2", target_bir_lowering=False, debug=False)

    xT = nc.dram_tensor("xT", [C, BT], BF16, kind="ExternalInput").ap()
    wqkvT = nc.dram_tensor("wqkvT", [C, 3 * 128], BF16, kind="ExternalInput").ap()
    bqkv_s = nc.dram_tensor("bqkv_s", [3, 128], F32, kind="ExternalInput").ap()
    woutT = nc.dram_tensor("woutT", [128, C], BF16, kind="ExternalInput").ap()
    ident = nc.dram_tensor("ident", [128, 128], BF16, kind="ExternalInput").ap()
    cmask = nc.dram_tensor("cmask", [128, 128], BF16, kind="ExternalInput").ap()
    genb = nc.dram_tensor("genb", [max(n_gen, 1), KT, QB], BF16, kind="ExternalInput").ap()
    partial = nc.dram_tensor("partial", [BT, C], F32, kind="ExternalOutput").ap()

    with tile.TileContext(nc) as tc:
        _emit(tc, plan, xT, wqkvT, bqkv_s, woutT, ident, cmask, genb,
              partial, loop_n=loop_n, phases=phases)
    nc.compile()
    return nc


def _emit(tc, plan, xT, wqkvT, bqkv_s, woutT, ident, cmask, genb,
          partial, loop_n=1, phases=("proj", "attn", "out")):
    from contextlib import ExitStack

    nc = tc.nc
    ctx = ExitStack()
    const = ctx.enter_context(tc.tile_pool(name="const", bufs=1))
    xin = ctx.enter_context(tc.tile_pool(name="xin", bufs=2))
    qkv = ctx.enter_context(tc.tile_pool(name="qkv", bufs=4))
    vpp = ctx.enter_context(tc.tile_pool(name="vp", bufs=4))
    ptile_pool = ctx.enter_context(tc.tile_pool(name="ptile", bufs=1))
    small = ctx.enter_context(tc.tile_pool(name="small", bufs=4))
    evac = ctx.enter_context(tc.tile_pool(name="evac", bufs=2))
    gbuf = ctx.enter_context(tc.tile_pool(name="gbuf", bufs=2))
    # PSUM budget, 8 banks of [128, 512] f32:
    #   ps_pm: 2 (proj accumulators / out-proj, tags pm0/pm1)
    #   ps_s:  4 (paired-head S^T [128, 1024] x 2 bufs)
    #   ps_o:  2 (PV accumulator per head)
    ps_pm = ctx.enter_context(tc.tile_pool(name="ps_pm", bufs=1, space="PSUM"))
    ps_s = ctx.enter_context(tc.tile_pool(name="ps_s", bufs=1, space="PSUM"))
    ps_o = ctx.enter_context(tc.tile_pool(name="ps_o", bufs=1, space="PSUM"))

    # ---- constants ----
    # single HWDGE DMA for the proj weights: the startup critical path is
    # per-DMA fixed overhead, not bytes
    w_sb = const.tile([128, C // 128, 384], BF16, tag="w_sb")
    nc.sync.dma_start(w_sb[:], wqkvT.rearrange("(ko p) m -> p ko m", p=128))
    bias_sb = const.tile([128, 3], F32, tag="bias_sb")
    nc.gpsimd.dma_start(bias_sb[:], bqkv_s.rearrange("m p -> p m"))
    wout_sb = const.tile([128, C], BF16, tag="wout_sb")
    ident_sb = const.tile([128, 128], BF16, tag="ident_sb")
    cmask_sb = const.tile([128, 128], BF16, tag="cmask_sb")

    def load_big_consts():
        nc.gpsimd.dma_start(ident_sb[:], ident)
        nc.gpsimd.dma_start(cmask_sb[:], cmask)
        nc.gpsimd.dma_start(wout_sb[:], woutT)

    nkc = C // 128
    TB = 512  # proj token-chunk; all 8 x-tiles of a chunk stay resident

    state = {}  # per-batch persistent tiles

    def proj_steps(b):
        """qkv projection + V'-build for batch b; yields after each chunk."""
        qT = qkv.tile([128, T], BF16, tag="qT", name="qT")
        kTt = qkv.tile([128, T], BF16, tag="kT", name="kTt")
        vT = qkv.tile([128, T], BF16, tag="vT", name="vT")
        vp_all = vpp.tile([128, NKT, HPC, 128], BF16, tag="vp", name="vp")
        # ones column for the softmax-denominator trick; V' columns [0:D]
        # are fully written by the DMA transposes below.
        nc.gpsimd.memset(vp_all[:, :, :, D : D + 1], 1.0)
        state[b] = dict(qT=qT, kTt=kTt, vT=vT, vp=vp_all)
        for jt in range(T // TB):
            tok0 = b * T + jt * TB
            sl = slice(jt * TB, (jt + 1) * TB)
            xch = xin.tile([128, nkc, TB], BF16, tag="xch", name="xch")
            nc.sync.dma_start(
                xch[:],
                xT[:, tok0 : tok0 + TB].rearrange("(ko p) t -> p ko t", p=128),
            )
            if b == 0 and jt == 0:
                load_big_consts()
            yield
            # q -> pm0, k -> pm1, v -> pm0 (second allocation): evacuation of
            # one group overlaps the next group's matmuls in the other bank.
            for m, (dest, tag) in enumerate(
                ((qT, "pm0"), (kTt, "pm1"), (vT, "pm0"))
            ):
                pm = ps_pm.tile([128, TB], F32, tag=tag, name="pm")
                for kc in range(nkc):
                    nc.tensor.matmul(
                        pm[:],
                        w_sb[:, kc, m * 128 : (m + 1) * 128],
                        xch[:, kc],
                        start=(kc == 0),
                        stop=(kc == nkc - 1),
                    )
                    if kc % 4 == 3:
                        yield
                nc.vector.tensor_scalar_add(
                    dest[:, sl], pm[:], bias_sb[:, m : m + 1]
                )
            # V' for this chunk's k-tiles: PE transpose (both heads at once),
            # then one DVE copy into the padded per-head layout. Keeping this
            # off the HWDGE queue matters: every hardware-DGE DMA costs ~625ns
            # serialized, and bursts of them head-of-line block the queue.
            for kt in range(jt * (TB // KT), (jt + 1) * (TB // KT)):
                pst = ps_pm.tile([128, 128], BF16, tag="pm1", name="pst")
                nc.tensor.transpose(
                    pst[:], vT[:, kt * KT : (kt + 1) * KT], ident_sb[:]
                )
                nc.vector.tensor_copy(
                    vp_all[:, kt, :, 0:D],
                    pst[:].rearrange("p (h d) -> p h d", h=HPC),
                )
                yield

    def attn_steps(b):
        """attention + out-projection for batch b; yields after each k-tile."""
        st = state[b]
        qT, kTt, vp_all = st["qT"], st["kTt"], st["vp"]
        oT = qkv.tile([128, T], BF16, tag="oT", name="oT")
        st["oT"] = oT
        chain = 0  # rotates psum/pt slots across per-head chains
        for j in range(NQB):
            tiles = plan[j]
            if not tiles:
                continue
            o_ps = [
                ps_o.tile([128, QB], F32, tag=f"o{hh}", name=f"o_ps{hh}")
                for hh in range(HPC)
            ]
            def emit_pv(item):
                kt_, off_, hh_, pt_, first_, last_ = item
                nc.tensor.matmul(
                    o_ps[hh_][0 : D + 1, off_:QB],
                    vp_all[:, kt_, hh_, 0 : D + 1],
                    pt_[:, off_:QB],
                    start=first_,
                    stop=last_,
                )

            pending = []
            for idx, (kt, typ, aux) in enumerate(tiles):
                first, last = idx == 0, idx == len(tiles) - 1
                # off = width of the fully-masked q-prefix of this tile
                # (cols [0, off) have every k masked -> never computed).
                off = aux if typ == CAUSAL else 0
                gb = None
                if typ == GENERAL:
                    gb = gbuf.tile([128, QB], BF16, tag="gb", name="gb")
                    nc.sync.dma_start(gb[:], genb[aux])
                # per-head chains: independent S -> exp -> PV pipelines hide
                # the cross-engine semaphore latency (measured 2.1x on HW vs
                # a paired-head chain)
                for hh in range(HPC):
                    p0 = hh * D
                    sp = ps_s.tile([128, QB], F32, tag=f"sp{chain % 4}", name="sp")
                    nc.tensor.matmul(
                        sp[:, off:QB],
                        kTt[p0 : p0 + D, kt * KT : (kt + 1) * KT],
                        qT[p0 : p0 + D, j * QB + off : (j + 1) * QB],
                        start=True,
                        stop=True,
                    )
                    pt = ptile_pool.tile([128, QB], BF16, tag=f"pt{chain % 8}",
                                         name="pt")
                    if typ == GENERAL:
                        nc.vector.tensor_add(pt[:], sp[:], gb[:])
                        nc.scalar.activation(
                            pt[:], pt[:],
                            mybir.ActivationFunctionType.Exp, scale=SCALE,
                        )
                    else:
                        nc.scalar.activation(
                            pt[:, off:QB],
                            sp[:, off:QB],
                            mybir.ActivationFunctionType.Exp,
                            scale=SCALE,
                        )
                        if typ == CAUSAL:
                            # zero the still-masked triangle inside the window
                            nc.vector.tensor_mul(
                                pt[:, off : off + 128],
                                pt[:, off : off + 128],
                                cmask_sb[:],
                            )
                    if len(pending) >= 4:
                        emit_pv(pending.pop(0))
                    pending.append((kt, off, hh, pt, first, last))
                    chain += 1
                yield
            for item in pending:
                emit_pv(item)
            r1s, r64s = [], []
            for hh in range(HPC):
                r1 = small.tile([1, QB], F32, tag=f"r1{hh}", name="r1")
                nc.vector.reciprocal(r1[:], o_ps[hh][D : D + 1, :])
                r1s.append(r1)
            for hh in range(HPC):
                r64 = small.tile([D, QB], F32, tag=f"r64{hh}", name="r64")
                nc.gpsimd.partition_broadcast(r64[:], r1s[hh][:])
                r64s.append(r64)
            for hh in range(HPC):
                nc.vector.tensor_mul(
                    oT[hh * D : (hh + 1) * D, j * QB : (j + 1) * QB],
                    o_ps[hh][0:D, :],
                    r64s[hh][:],
                )
            if "out" not in phases:
                continue
            # out-projection for the token rows finalized by this q-block;
            # one ev tile + one partial-out DMA per q-block (fewer HWDGE ops)
            ev = evac.tile([128, 4, C], F32, tag="ev", name="ev")
            for ti in range(4):
                tt = 4 * j + ti
                for n in range(C // QB):
                    po = ps_pm.tile([128, QB], F32, tag=f"pm{n % 2}", name="po")
                    nc.tensor.matmul(
                        po[:],
                        oT[:, tt * 128 : (tt + 1) * 128],
                        wout_sb[:, n * QB : (n + 1) * QB],
                        start=True,
                        stop=True,
                    )
                    nc.vector.tensor_copy(
                        ev[:, ti, n * QB : (n + 1) * QB], po[:]
                    )
                yield
            r0 = b * T + j * QB
            nc.sync.dma_start(
                partial[r0 : r0 + QB, :].rearrange("(ti p) c -> p ti c", p=128),
                ev[:],
            )

    # ---- phase-separated schedule: the attn chain is already PE-bound
    # (4 matmuls/tile ~ chain rate), so interleaving proj into it only
    # breaks the in-order PE queue pipelining; run all projections first.
    loop_cm = tc.For_i(0, loop_n, 1) if loop_n > 1 else None
    if loop_cm is not None:
        loop_cm.__enter__()
    for bb in range(B):
        for _ in proj_steps(bb):
            pass
    for b in range(B):
        if "attn" not in phases:
            # phase-bisection mode: dump proj outputs for offline check
            st = state[b]
            dbg = evac.tile([128, 2, C], F32, tag="ev", name="dbg")
            nc.vector.tensor_copy(dbg[:, 0, 0:256], st["qT"][:, 0:256])
            nc.vector.tensor_copy(dbg[:, 0, 256:512], st["kTt"][:, 0:256])
            nc.vector.tensor_copy(dbg[:, 0, 512:768], st["vT"][:, 0:256])
            nc.vector.tensor_copy(
                dbg[:, 0, 768:768 + 256],
                st["vp"][:, 0].rearrange("p h d -> p (h d)"),
            )
            nc.scalar.dma_start(
                partial[b * T : b * T + 128, :], dbg[:, 0, :]
            )
            continue
        for _ in attn_steps(b):
            pass
        if "out" not in phases:
            st = state[b]
            dbg = evac.tile([128, QB], F32, tag="ev", name="dbg2")
            nc.vector.tensor_copy(dbg[:, 0:4], st["oT"][:, 0:4])
            nc.scalar.dma_start(partial[b : b + 1, 0:128], dbg[0:1, 0:128])

    if loop_cm is not None:
        loop_cm.__exit__(None, None, None)
    ctx.close()


def _prep_inputs(x, mask, Wqkv, bqkv, Wout):
    x = np.asarray(x, np.float32)
    Wqkv = np.asarray(Wqkv, np.float32)
    bqkv = np.asarray(bqkv, np.float32)
    Wout = np.asarray(Wout, np.float32)
    mask2d = np.asarray(mask).reshape(T, T)

    plan, genbias = _classify_mask(mask2d)

    xT = np.ascontiguousarray(x.reshape(BT, C).T).astype(BF16NP)
    ident = np.eye(128, dtype=np.float32).astype(BF16NP)
    cmask = np.triu(np.ones((128, 128), np.float32)).astype(BF16NP)
    genbias_bf = genbias.astype(BF16NP)

    in_maps = []
    for core in range(NCORES):
        h0 = core * HPC * D  # first q-row of this core's heads
        wq = Wqkv[h0 : h0 + 128, :]
        wk = Wqkv[C + h0 : C + h0 + 128, :]
        wv = Wqkv[2 * C + h0 : 2 * C + h0 + 128, :]
        wqkvT = np.ascontiguousarray(np.concatenate([wq, wk, wv], 0).T).astype(BF16NP)
        bq = np.stack([bqkv[h0 : h0 + 128], bqkv[C + h0 : C + h0 + 128],
                       bqkv[2 * C + h0 : 2 * C + h0 + 128]])
        woutT = np.ascontiguousarray(Wout[:, h0 : h0 + 128].T).astype(BF16NP)
        in_maps.append({
            "xT": xT,
            "wqkvT": wqkvT,
            "bqkv_s": np.ascontiguousarray(bq.astype(np.float32)),
            "woutT": woutT,
            "ident": ident,
            "cmask": cmask,
            "genb": genbias_bf,
        })
    return plan, genbias, in_maps


def run(x, mask, Wqkv, bqkv, Wout, bout, trace=False, trace_kwargs=None):
    plan, genbias, in_maps = _prep_inputs(x, mask, Wqkv, bqkv, Wout)
    nc = _build_program(plan, genbias.shape[0])
    res = run_bass_kernel_spmd(
        nc,
        in_maps,
        core_ids=list(range(NCORES)),
        trace=trace,
        **(trace_kwargs or {}),
    )
    acc = np.zeros((BT, C), np.float64)
    for core in range(NCORES):
        acc += res.results[core]["partial"].astype(np.float64)
    out = (acc + np.asarray(bout, np.float64)).astype(np.float32)
    return out.reshape(B, T, C), res


def kernel(x, mask, Wqkv, bqkv, Wout, bout):
    out, _ = run(x, mask, Wqkv, bqkv, Wout, bout, trace=False)
    return out
